# revision 53
# baseline (speedup 1.0000x reference)
"""Trainium2 distributed kernel for the multi-query sparse-attention block.

Sharding: 8 cores = 2 batches x 4 head-groups (4 heads each).
J (key/value axis) is host-permuted to [self(2048) | ctx(256) | null(1) | pad(127)]
and the attention bias arrives pre-transposed (j-major), mask-folded and
pre-exponentiated in bf16:  attn_weight = exp(q.k) * expb.
Softmax runs without max-subtraction; the denominator comes from a ones-column
appended to V.  Output projection partials are ReduceScattered over each
4-core batch group, and the final layernorm runs on the scattered shards.

v2: DMA-transposes replace PE transposes; exp reads 3-bank PSUM groups;
only exp/ln activations (single table set); batched eb DMAs; DVE
reciprocal for softmax denominators; final-LN interleaved with attention.
"""

import sys

sys.path.insert(0, "/opt/trn_rl_repo")

import numpy as np
import ml_dtypes

import concourse.bass as bass
import concourse.mybir as mybir
import concourse.tile as tile
from concourse import bacc
from concourse.bass_utils import run_bass_kernel_spmd

F32 = mybir.dt.float32
F32R = mybir.dt.float32r
BF16 = mybir.dt.bfloat16
AF = mybir.ActivationFunctionType
ALU = mybir.AluOpType



B, N, D = 2, 2048, 1024
H, DH = 16, 64
C, CD = 256, 512
J = C + 1 + N          # 2305
JP = 19 * 128          # 2432 padded
NB = 19                # j blocks
HPC = 4                # heads per core
EPS = 1e-5

_cache = {}


def build():
    nc = bacc.Bacc("TRN2", target_bir_lowering=False, debug=False, num_devices=8)

    expb = nc.declare_dram_parameter("expb", [HPC, JP, N], BF16, isOutput=False)
    x_in = nc.declare_dram_parameter("x", [N, D], F32, isOutput=False)
    ctx_in = nc.declare_dram_parameter("ctxt", [C, CD], F32, isOutput=False)
    nullk = nc.declare_dram_parameter("nullk", [DH, 1], F32, isOutput=False)
    nullv = nc.declare_dram_parameter("nullv", [1, DH], F32, isOutput=False)
    wq_in = nc.declare_dram_parameter("wq", [D, 256], BF16, isOutput=False)
    wkv_in = nc.declare_dram_parameter("wkv", [D, 128], BF16, isOutput=False)
    wctx_in = nc.declare_dram_parameter("wctx", [CD, 128], BF16, isOutput=False)
    bctx_in = nc.declare_dram_parameter("bctx2", [1, 128], BF16, isOutput=False)
    wout_in = nc.declare_dram_parameter("wout", [256, D], BF16, isOutput=False)
    outg_in = nc.declare_dram_parameter("outg", [1, D], F32, isOutput=False)
    out_ext = nc.declare_dram_parameter("out", [N // 4, D], F32, isOutput=True)

    rs_in = [nc.dram_tensor(f"rs_in{c}", [512, D], BF16) for c in range(4)]
    rs_out = [nc.dram_tensor(f"rs_out{c}", [128, D], BF16) for c in range(4)]

    with tile.TileContext(nc) as tc:
        with tc.tile_pool(name="persist", bufs=1) as pp:
            wq_r = pp.tile([128, 8, 256], BF16)
            wkv_r = pp.tile([128, 8, 128], BF16)
            wctx_r = pp.tile([128, 4, 128], BF16)
            wout_r = pp.tile([128, 2, 1024], BF16)
            bctx_r = pp.tile([1, 128], BF16)
            ones_r = pp.tile([1, 256], BF16)
            gamma_bc = pp.tile([128, 1024], F32)
            qT = pp.tile([64, HPC * N], BF16)
            kT = pp.tile([64, JP], BF16)
            vext = pp.tile([128, NB * 65], BF16)
            aoT0 = pp.tile([128, N], BF16)
            aoT1 = pp.tile([128, N], BF16)
            aoT = [aoT0, aoT1]
            eps_t = pp.tile([128, 1], F32)

            # ---- startup DMAs: x tiles first, then ctx ----
            xt_pool = tc.tile_pool(name="xt", bufs=16)
            xp = xt_pool.__enter__()
            xtiles = []
            for it in range(16):
                xt = xp.tile([128, D], F32, tag="xt", name=f"xt{it}")
                xtiles.append(xt)
            nc.sync.dma_start(out=wctx_r[:], in_=wctx_in.rearrange("(c p) f -> p c f", p=128))
            nc.sync.dma_start(out=bctx_r[:], in_=bctx_in[:])
            nc.sync.dma_start(out=wq_r[:], in_=wq_in.rearrange("(c p) f -> p c f", p=128))
            nc.sync.dma_start(out=wkv_r[:], in_=wkv_in.rearrange("(c p) f -> p c f", p=128))
            for it in range(4):
                nc.sync.dma_start(out=xtiles[it][:],
                                  in_=x_in[it * 128:(it + 1) * 128, :])
            ct_pool = tc.tile_pool(name="cw", bufs=2)
            cw = ct_pool.__enter__()
            cts = []
            for t in range(2):
                ct = cw.tile([128, CD], F32, tag="ct", name=f"ct{t}")
                nc.sync.dma_start(out=ct[:], in_=ctx_in[t * 128:(t + 1) * 128, :])
                cts.append(ct)
            for it in range(4, 16):
                nc.sync.dma_start(out=xtiles[it][:],
                                  in_=x_in[it * 128:(it + 1) * 128, :])
            nc.gpsimd.dma_start(out=kT[:, 2304:2305], in_=nullk[:])

            nc.gpsimd.memset(eps_t[:], EPS)
            zrow = pp.tile([64, 127], F32)
            nc.vector.memset(zrow[:], 0.0)
            nc.scalar.copy(kT[:, 2305:2432], zrow[:])
            nc.vector.memset(vext[:, 18 * 65:18 * 65 + 64], 0.0)
            nc.gpsimd.dma_start(out=vext[0:1, 18 * 65:18 * 65 + 64], in_=nullv[:])
            for jb in range(NB):
                nc.vector.memset(vext[:, jb * 65 + 64:jb * 65 + 65], 1.0)
            o1 = pp.tile([1, 256], F32)
            nc.vector.memset(o1[:], 1.0)
            nc.scalar.copy(ones_r[:], o1[:])
            og_sb = pp.tile([1, 1024], F32)
            nc.sync.dma_start(out=og_sb[:], in_=outg_in[:])
            nc.gpsimd.partition_broadcast(gamma_bc[:], og_sb[:])

            def ln_rstd_negm(pool, mv, k, tagp):
                """mv: [128, k, 2] (mean, var) -> rstd, negm [128, k] via ln/exp."""
                lnv = pool.tile([128, k], F32, tag=f"{tagp}lnv")
                nc.scalar.activation(lnv[:], mv[:, :, 1], AF.Ln, bias=eps_t[:, :])
                rstd = pool.tile([128, k], F32, tag=f"{tagp}rstd")
                nc.scalar.activation(rstd[:], lnv[:], AF.Exp, scale=-0.5)
                negm = pool.tile([128, k], F32, tag=f"{tagp}negm")
                nc.vector.tensor_scalar_mul(out=negm[:], in0=mv[:, :, 0], scalar1=-1.0)
                return rstd, negm

            # ---------------- context tokens -> kT / vext ----------------
            with tc.tile_pool(name="cstat", bufs=2) as cs, \
                 tc.tile_pool(name="cps", bufs=2, space="PSUM") as cps:
                cnT = pp.tile([128, 2, 4, 128], BF16)  # [p, t, c, tok]
                cmv = cs.tile([128, 2, 2], F32, tag="cmv")
                for t in range(2):
                    cstat = cs.tile([128, 1, 6], F32, tag="cst", name=f"cst{t}")
                    nc.vector.bn_stats(out=cstat[:, 0, :], in_=cts[t][:])
                    nc.vector.bn_aggr(out=cmv[:, t, :], in_=cstat[:, :, :])
                crstd, cnegm = ln_rstd_negm(cs, cmv, 2, "c")
                for t in range(2):
                    cn = cw.tile([128, CD], BF16, tag="cn", name=f"cn{t}")
                    nc.vector.tensor_scalar(
                        out=cn[:], in0=cts[t][:],
                        scalar1=cnegm[:, t:t + 1], scalar2=crstd[:, t:t + 1],
                        op0=ALU.add, op1=ALU.mult)
                    nc.scalar.dma_start(
                        out=cnT[:, t], in_=cn[:], transpose=True)
                pck = cps.tile([64, 256], F32, tag="ck")
                for c in range(4):
                    nc.tensor.matmul(pck[:], wctx_r[:, c, 0:64], cnT[:, :, c, :],
                                     start=(c == 0), stop=False)
                nc.tensor.matmul(pck[:], bctx_r[:, 0:64], ones_r[:, 0:256],
                                 start=False, stop=True)
                nc.scalar.copy(kT[:, 2048:2304], pck[:])
                for t in range(2):
                    pcv = cps.tile([128, 64], F32, tag="cv", name=f"cv{t}")
                    for c in range(4):
                        nc.tensor.matmul(pcv[:], cnT[:, t, c, :],
                                         wctx_r[:, c, 64:128],
                                         start=(c == 0), stop=False)
                    nc.tensor.matmul(pcv[:], ones_r[:, 0:128], bctx_r[:, 64:128],
                                     start=False, stop=True)
                    nc.vector.tensor_copy(vext[:, (16 + t) * 65:(16 + t) * 65 + 64],
                                          pcv[:])
            ct_pool.__exit__(None, None, None)

            # ---------------- x: LN + DMA-transpose + projections ----------------
            nc.sync.dma_start(
                out=wout_r[:], in_=wout_in.rearrange("(c p) f -> p c f", p=128))
            with tc.tile_pool(name="xst", bufs=2) as xs, \
                 tc.tile_pool(name="xnt", bufs=2) as xnp, \
                 tc.tile_pool(name="vtmp", bufs=2) as vtp, \
                 tc.tile_pool(name="xps", bufs=2, space="PSUM") as xps, \
                 tc.tile_pool(name="pps", bufs=2, space="PSUM") as pps:
                for ic in range(4):
                    xts = xtiles[ic * 4:(ic + 1) * 4]
                    xmv = xs.tile([128, 4, 2], F32, tag="xmv")
                    for tb in range(4):
                        xst = xs.tile([128, 2, 6], F32, tag="xst", name=f"xst{tb}")
                        xr = xts[tb][:].rearrange("p (n f) -> p n f", f=512)
                        for s in range(2):
                            nc.vector.bn_stats(out=xst[:, s, :], in_=xr[:, s, :])
                        nc.vector.bn_aggr(out=xmv[:, tb, :], in_=xst[:, :, :])
                    xrstd, xnegm = ln_rstd_negm(xs, xmv, 4, "x")
                    xnT = xnp.tile([128, 4, 8, 128], BF16, tag="xnT")  # [p, tb, c, tok]
                    for tb in range(4):
                        xn = xnp.tile([128, D], BF16, tag="xn", name=f"xn{tb}")
                        nc.vector.tensor_scalar(
                            out=xn[:], in0=xts[tb][:],
                            scalar1=xnegm[:, tb:tb + 1], scalar2=xrstd[:, tb:tb + 1],
                            op0=ALU.add, op1=ALU.mult)
                        nc.scalar.dma_start(
                            out=xnT[:, tb], in_=xn[:], transpose=True)
                    for m in range(2):
                        pq = pps.tile([128, 512], F32, tag="pq", name=f"pq{m}")
                        for c in range(8):
                            nc.tensor.matmul(pq[:], wq_r[:, c, m * 128:(m + 1) * 128],
                                             xnT[:, :, c, :],
                                             start=(c == 0), stop=(c == 7))
                        for hh in range(2):
                            h = 2 * m + hh
                            nc.scalar.copy(
                                qT[:, h * N + ic * 512:h * N + ic * 512 + 512],
                                pq[hh * 64:hh * 64 + 64, :])
                    pkv = pps.tile([128, 512], F32, tag="pkv")
                    for c in range(8):
                        nc.tensor.matmul(pkv[:], wkv_r[:, c, :], xnT[:, :, c, :],
                                         start=(c == 0), stop=(c == 7))
                    nc.scalar.copy(kT[:, ic * 512:ic * 512 + 512], pkv[0:64, :])
                    vt = vtp.tile([64, 512], BF16, tag="vt")
                    nc.vector.tensor_copy(vt[:], pkv[64:128, :])
                    vtr = vtp.tile([128, 4, 64], BF16, tag="vtr")
                    nc.scalar.dma_start(out=vtr[:], in_=vt[:], transpose=True)
                    nc.vector.tensor_copy(
                        vext[:].rearrange("p (b f) -> p b f", f=65)
                            [:, ic * 4:(ic + 1) * 4, 0:64],
                        vtr[:])
            xt_pool.__exit__(None, None, None)

            # ---------------- attention + interleaved out-proj/RS/LN ----------------
            with tc.tile_pool(name="eb", bufs=4) as ebp, \
                 tc.tile_pool(name="et", bufs=6) as etp, \
                 tc.tile_pool(name="nrm", bufs=2) as nrm, \
                 tc.tile_pool(name="ysb", bufs=3) as yp, \
                 tc.tile_pool(name="fln", bufs=1) as flp, \
                 tc.tile_pool(name="fst", bufs=2) as fs, \
                 tc.tile_pool(name="psA", bufs=1, space="PSUM") as psA, \
                 tc.tile_pool(name="psB", bufs=1, space="PSUM") as psB, \
                 tc.tile_pool(name="pso", bufs=2, space="PSUM") as pso, \
                 tc.tile_pool(name="psy", bufs=2, space="PSUM") as psy:

                eb_tiles = {}

                def fetch_eb(h, iq):
                    t = ebp.tile([128, NB, 512], BF16, tag="eb", name=f"eb{h}_{iq}")
                    nc.sync.dma_start(
                        out=t[:],
                        in_=expb[h].rearrange("(g p) i -> p g i", p=128)
                            [:, :, iq * 512:(iq + 1) * 512])
                    eb_tiles[(h, iq)] = t

                # prefetch first three eb tiles (order: h-major within iq)
                fetch_eb(0, 0)
                fetch_eb(1, 0)
                fetch_eb(2, 0)
                fetch_eb(3, 0)

                def out_block(ib):
                    y = yp.tile([128, 1024], BF16, tag="y")
                    for ec in range(2):
                        py = psy.tile([128, 512], F32, tag="py", name=f"py{ec}")
                        for c in range(2):
                            nc.tensor.matmul(py[:],
                                             aoT[c][:, ib * 128:(ib + 1) * 128],
                                             wout_r[:, c, ec * 512:(ec + 1) * 512],
                                             start=(c == 0), stop=(c == 1))
                        nc.vector.tensor_copy(y[:, ec * 512:(ec + 1) * 512], py[:])
                    ch = ib // 4
                    nc.gpsimd.dma_start(
                        out=rs_in[ch][(ib % 4) * 128:(ib % 4 + 1) * 128, :], in_=y[:])

                def issue_rs(ch):
                    nc.gpsimd.collective_compute(
                        "ReduceScatter", ALU.add,
                        replica_groups=[[0, 1, 2, 3], [4, 5, 6, 7]],
                        ins=[rs_in[ch][:]], outs=[rs_out[ch][:]])

                def final_ln(ch):
                    ft = flp.tile([128, 1024], BF16, tag="ft")
                    nc.gpsimd.dma_start(out=ft[:], in_=rs_out[ch][:])
                    fmv = fs.tile([128, 1, 2], F32, tag="fmv")
                    fst = fs.tile([128, 2, 6], F32, tag="fst")
                    fr = ft[:].rearrange("p (n f) -> p n f", f=512)
                    for s in range(2):
                        nc.vector.bn_stats(out=fst[:, s, :], in_=fr[:, s, :])
                    nc.vector.bn_aggr(out=fmv[:, 0, :], in_=fst[:, :, :])
                    frstd, fnegm = ln_rstd_negm(fs, fmv, 1, "f")
                    fn = flp.tile([128, 1024], F32, tag="fn")
                    nc.vector.tensor_scalar(
                        out=fn[:], in0=ft[:],
                        scalar1=fnegm[:, 0:1], scalar2=frstd[:, 0:1],
                        op0=ALU.add, op1=ALU.mult)
                    nc.vector.tensor_tensor(out=fn[:], in0=fn[:], in1=gamma_bc[:],
                                            op=ALU.mult)
                    nc.gpsimd.dma_start(out=out_ext[ch * 128:(ch + 1) * 128, :],
                                        in_=fn[:])

                NG = 10                      # j-block groups: 9 of 2 + 1 of 1
                gsz = lambda g: 2 if g < 9 else 1
                pending_norm = [None]

                def emit_norm():
                    # normalize the previous head's po: 1/den via ln/exp,
                    # broadcast over partitions with a K=1 PE matmul.
                    if pending_norm[0] is None:
                        return
                    po, h, piq = pending_norm[0]
                    pending_norm[0] = None
                    lg = nrm.tile([1, 512], F32, tag="lg")
                    nc.scalar.activation(lg[:], po[64:65, :], AF.Ln)
                    rec = nrm.tile([1, 512], F32, tag="rec")
                    nc.scalar.activation(rec[:], lg[:], AF.Exp, scale=-1.0)
                    rbc = nrm.tile([64, 512], F32, tag="rbc")
                    nc.gpsimd.partition_broadcast(rbc[:], rec[:])
                    nc.vector.tensor_tensor(
                        out=aoT[h // 2][(h % 2) * 64:(h % 2) * 64 + 64,
                                        piq * 512:(piq + 1) * 512],
                        in0=po[0:64, :], in1=rbc[:], op=ALU.mult)

                for iq in range(4):
                    for h in range(HPC):
                        eb = eb_tiles.pop((h, iq))
                        nh, niq = (h + 4) % HPC, iq + (h + 4) // HPC
                        if niq < 4:
                            fetch_eb(nh, niq)
                        po = pso.tile([65, 512], F32, tag="po")
                        pss = {}
                        ets = {}

                        def emit_sim(g):
                            gs = gsz(g)
                            pool = (psA, psB)[g % 2]
                            ps = pool.tile([128, 2, 512], F32, tag="ps",
                                           name=f"ps{g % 2}")
                            for k in range(gs):
                                jb = 2 * g + k
                                nc.tensor.matmul(
                                    ps[:, k, :],
                                    kT[:, jb * 128:(jb + 1) * 128],
                                    qT[:, h * N + iq * 512:h * N + iq * 512 + 512],
                                    start=True, stop=True)
                            pss[g] = ps

                        def emit_exp_mult(g):
                            gs = gsz(g)
                            ps = pss.pop(g)
                            et = etp.tile([128, 2, 512], BF16, tag="et",
                                          name=f"et{g % 6}")
                            nc.scalar.activation(
                                et[:, 0:gs, :].rearrange("p a b -> p (a b)"),
                                ps[:, 0:gs, :].rearrange("p a b -> p (a b)"),
                                AF.Exp)
                            nc.vector.tensor_tensor(
                                out=et[:, 0:gs, :].rearrange("p a b -> p (a b)"),
                                in0=et[:, 0:gs, :].rearrange("p a b -> p (a b)"),
                                in1=eb[:, 2 * g:2 * g + gs, :].rearrange(
                                    "p a b -> p (a b)"),
                                op=ALU.mult)
                            ets[g] = et

                        def emit_av(g):
                            et = ets.pop(g)
                            for k in range(gsz(g)):
                                jb = 2 * g + k
                                nc.tensor.matmul(
                                    po[:],
                                    vext[:, jb * 65:jb * 65 + 65],
                                    et[:, k, :],
                                    start=(jb == 0), stop=(jb == NB - 1))

                        for g in range(NG + 4):
                            if g < NG:
                                emit_sim(g)
                                emit_exp_mult(g)
                            if g == 2:
                                emit_norm()   # previous head, after 3 sim groups
                            if g >= 4:
                                emit_av(g - 4)

                        pending_norm[0] = (po, h, iq)
                    emit_norm()               # last head before the out-proj
                    for ibl in range(4):
                        out_block(iq * 4 + ibl)
                    issue_rs(iq)
                    if iq >= 2:
                        final_ln(iq - 2)
                final_ln(2)
                final_ln(3)

    # Restrict exp/ln to the combined natural_log_exp_and_others table set so
    # the table-load pass keeps one set resident instead of alternating.
    orig_get_tables = bacc.get_activation_tables

    def _tables_one_set(arch):
        tabs = orig_get_tables(arch)
        exp_fn = AF.Exp if hasattr(AF, "Exp") else None
        out = {}
        for name, fns in tabs.items():
            if name != "natural_log_exp_and_others":
                fns = {f for f in fns
                       if str(f).lower().split(".")[-1] not in ("exp", "ln")}
            out[name] = fns
        return out

    bacc.get_activation_tables = _tables_one_set
    try:
        nc.compile()
    finally:
        bacc.get_activation_tables = orig_get_tables
    return nc


def _prep(inputs):
    x = np.asarray(inputs["x"], dtype=np.float32)
    context = np.asarray(inputs["context"], dtype=np.float32)
    mask = np.asarray(inputs["mask"])
    ab = np.asarray(inputs["attn_bias"], dtype=np.float32)
    norm_gamma = np.asarray(inputs["norm_gamma"], dtype=np.float32)
    null_kv = np.asarray(inputs["null_kv"], dtype=np.float32)
    Wq = np.asarray(inputs["Wq"], dtype=np.float32)
    Wkv = np.asarray(inputs["Wkv"], dtype=np.float32)
    ctx_ln_w = np.asarray(inputs["ctx_ln_w"], dtype=np.float32)
    ctx_ln_b = np.asarray(inputs["ctx_ln_b"], dtype=np.float32)
    Wctx = np.asarray(inputs["Wctx"], dtype=np.float32)
    bctx = np.asarray(inputs["bctx"], dtype=np.float32)
    Wout = np.asarray(inputs["Wout"], dtype=np.float32)
    out_gamma = np.asarray(inputs["out_gamma"], dtype=np.float32)

    scale = DH ** -0.5
    wq_f = ((norm_gamma[:, None] * Wq) * scale).astype(ml_dtypes.bfloat16)
    wkv_f = np.ascontiguousarray(
        (norm_gamma[:, None] * Wkv).astype(ml_dtypes.bfloat16))
    wctx_f = np.ascontiguousarray(
        (ctx_ln_w[:, None] * Wctx).astype(ml_dtypes.bfloat16))
    bctx2 = np.ascontiguousarray(
        (ctx_ln_b @ Wctx + bctx)[None, :].astype(ml_dtypes.bfloat16))
    outg = np.ascontiguousarray(out_gamma[None, :])
    nullk = np.ascontiguousarray(null_kv[0][:, None])
    nullv = np.ascontiguousarray(null_kv[1][None, :])

    # J permute [self | ctx | null], transpose j-major, exponentiate
    bp = np.concatenate([ab[:, :, C + 1:], ab[:, :, :C + 1]], axis=2)
    ebT = np.exp(np.ascontiguousarray(bp.transpose(0, 2, 1)))  # (H, J, N) f32
    mvec = np.empty((B, J), dtype=np.float32)
    mvec[:, :N] = mask[:, C:]
    mvec[:, N] = 1.0                       # ctx[0]: the left-pad True
    mvec[:, N + 1:N + C] = mask[:, :C - 1]  # ctx[c] <- mask[c-1]
    mvec[:, N + C] = mask[:, C - 1]         # null <- mask[255]

    in_maps = []
    for core in range(8):
        b, g = core // 4, core % 4
        eb = ebT[HPC * g:HPC * g + HPC] * mvec[b][None, :, None]
        ebp = np.zeros((HPC, JP, N), dtype=ml_dtypes.bfloat16)
        ebp[:, :J, :] = eb.astype(ml_dtypes.bfloat16)
        in_maps.append({
            "expb": ebp,
            "x": np.ascontiguousarray(x[b]),
            "ctxt": np.ascontiguousarray(context[b]),
            "nullk": nullk,
            "nullv": nullv,
            "wq": np.ascontiguousarray(wq_f[:, 256 * g:256 * (g + 1)]),
            "wkv": wkv_f,
            "wctx": wctx_f,
            "bctx2": bctx2,
            "wout": np.ascontiguousarray(
                Wout[256 * g:256 * (g + 1), :].astype(ml_dtypes.bfloat16)),
            "outg": outg,
        })
    return in_maps


def kernel(**inputs) -> np.ndarray:
    if "nc" not in _cache:
        _cache["nc"] = build()
    nc = _cache["nc"]
    in_maps = _prep(inputs)
    res = run_bass_kernel_spmd(nc, in_maps, core_ids=list(range(8))).results
    out = np.empty((B, N, D), dtype=np.float32)
    for core in range(8):
        b, r = core // 4, core % 4
        o = res[core]["out"]
        for ch in range(4):
            out[b, 512 * ch + 128 * r:512 * ch + 128 * (r + 1), :] = \
                o[ch * 128:(ch + 1) * 128]
    return out
